# revision 34
# baseline (speedup 1.0000x reference)
"""ArcFace-style loss kernel for Trainium2 (8 NeuronCores).

Strategy (v2)
-------------
Per (b, m) row the loss needs ``sum_full = sum_c exp(SCALE * cos[b,m,c])``
over all 200k classes plus the cosine at the 4 label columns (exact, host).
Classes are sharded 8-way (25000/core, padded to 25088).

Device pipeline per core, per super-block of classes:
  * one fp8 DMA of W^T packed [96, 2, W] (d = t*96+k), prescaled by 8
  * DoubleRow fp8 matmuls (0.5 cyc/row): psum[128, W] = 64*cos in <=512 chunks
  * exp split across two engines, balanced so both run ~equal:
      - ACT: exact Exp(psum * 30/64) on the first ~58% cols, bf16 out,
        fused accum_out -> per-super partial sum (f32)
      - DVE: Schraudolph fast-exp on the rest: one tensor_scalar
        int16(round(psum * A + B)) whose bit pattern IS bf16(e^x) to ~±2.3%
        (bounded, input-independent), then a 4x-mode tensor_scalar accum of
        the bf16-bitcast view
  * partial sums land in acc[128, 2/super]; the whole acc tile is DMA'd out
    and reduced on host (shorter device tail than an on-device reduce).

Cost model: DMA 13.4us (fp8 stream, the floor), ACT ~16.6us, DVE ~16.6us,
PE ~6us. Schraudolph's ±2.3% per-element bound -> ~0.3% on row sums ->
~3e-3 absolute on log-sum-exp, vs ~0.55 tolerance: ~100x margin.

Host: l2-normalize, gather the 128 label rows for exact f64 cos_l, ArcFace +
Hungarian + BCE epilogue in f64 (unchanged from v1).
"""

import math
from contextlib import ExitStack

import numpy as np

import concourse.bass as bass
import concourse.tile as tile
from concourse import bacc, mybir
from concourse.bass_utils import run_bass_kernel_spmd

# ---- problem constants (hardcoded per contract) ----
B, M, D, NC = 32, 4, 192, 200000
BM = B * M                       # 128 rows
N_CORES = 8
C_SH = NC // N_CORES             # 25000 classes per core
S_SPK = 4
SCALE = 30.0
MARGIN = 0.5
ETA, XI = 2.5, 5.0
COS_M = math.cos(MARGIN)
SIN_M = math.sin(MARGIN)
TH = math.cos(math.pi - MARGIN)
MM = math.sin(math.pi - MARGIN) * MARGIN
EPS = 1e-6

# ---- kernel tiling ----
PRE = 8.0                        # fp8 prescale on both operands -> psum = 64*cos
ACT_SCALE = SCALE / (PRE * PRE)  # 30/64, exact in binary
# Schraudolph fast-exp (bf16 flavour): bf16_bits(e^x) ~ int16(round(x*A + B))
SCH_A = (2.0 ** 7) / math.log(2.0)          # 184.6650558
SCH_C = 7.36                                 # calibrated: zero-mean ratio
SCH_B = 127.0 * 128.0 - SCH_C                # 16248.64
SCH_A_EFF = SCH_A * ACT_SCALE                # applied to psum (=64*cos)
# exact device value of a padded (all-zero) class column on the DVE path:
# int16(round(16248.64)) = 16249 = 0x3F79, bitcast bf16 -> 0.97265625
SCH_ZERO = 0.97265625

# super-blocks: one DMA + two half-width consumer instructions each, whole
# supers alternating between ACT (exact Exp) and DVE (Schraudolph). Variable
# pair widths (A 2560/D 1536 vs A 2048/D 2048) balance the two engines;
# GPSIMD (otherwise idle) pairwise-folds the Schraudolph outputs so DVE's
# final 4x reduces touch half the data.
# pairs: (A-width, D-width); block0 = 512 on DVE
PAIRS = [(2048, 2048)] * 6
SUPER_WIDTHS = [512] + [2048] * 12                     # sum = 25088
# 7 ACT : 5 DVE supers balances ACT 1.196ns/col vs DVE ~1.45ns/col (incl red)
ASSIGN = "D" + "ADADADADADAA"
C_PAD = sum(SUPER_WIDTHS)                              # 25088
N_PAD = C_PAD - C_SH                                   # 88 zero cols, last D super
PAD_SUB = float(N_PAD) * SCH_ZERO                      # Schraudolph(0) each
W_BUFS = 8                                             # W stream runahead depth

DTYPE = "fp8dr"   # tag for the cache / test harness

LAST_EXEC_NS = None
LAST_RESULTS = None

_CACHE = {}


def _build():
    fp8 = mybir.dt.float8e4
    f32 = mybir.dt.float32
    bf16 = mybir.dt.bfloat16
    i16 = mybir.dt.int16
    AF = mybir.ActivationFunctionType

    n_out = 2 * sum(1 for c in ASSIGN if c == "A") + 4

    nc = bacc.Bacc(
        "TRN2", target_bir_lowering=False, debug=False, num_devices=N_CORES
    )
    wt = nc.dram_tensor("wt", [96, 2, C_PAD], fp8, kind="ExternalInput").ap()
    xt = nc.dram_tensor("xt", [96, 2, BM], fp8, kind="ExternalInput").ap()
    out = nc.dram_tensor("out", [BM, n_out], f32, kind="ExternalOutput").ap()

    with tile.TileContext(nc) as tc, ExitStack() as ctx:
        xp = ctx.enter_context(tc.tile_pool(name="x", bufs=1))
        wp = ctx.enter_context(tc.tile_pool(name="w", bufs=W_BUFS))
        pp = ctx.enter_context(tc.tile_pool(name="ps", bufs=1, space="PSUM"))
        ep = ctx.enter_context(tc.tile_pool(name="ex", bufs=2))
        sp = ctx.enter_context(tc.tile_pool(name="sx", bufs=1))
        gp = ctx.enter_context(tc.tile_pool(name="gf", bufs=1))
        dp = ctx.enter_context(tc.tile_pool(name="dd", bufs=2))
        accp = ctx.enter_context(tc.tile_pool(name="acc", bufs=1))

        xtile = xp.tile([96, 2, BM], fp8, tag="xt")
        nc.sync.dma_start(xtile[:], xt[:, :, :])

        n_act = 2 * sum(1 for c in ASSIGN if c == "A")   # A-halves
        n_red = 4                       # see red plan below
        acc = accp.tile([BM, n_act + n_red], f32, tag="acc")
        nc.vector.memset(acc[:], 0.0)
        # dummy 1-elem Exp pulls the activation-table load off the critical
        # path (overlaps the first W DMA)
        warm = accp.tile([BM, 1], f32, tag="warm")
        nc.gpsimd.memset(warm[:], 0.0)
        nc.scalar.activation(warm[:], warm[:], AF.Exp, bias=0.0, scale=0.0)

        # PE p-state warmup: dummy matmuls on a garbage tile keep the PE
        # busy from t~0.3us so the first real matmuls run at full clock
        wgarb = accp.tile([96, 2, 512], fp8, tag="wgarb")
        nc.gpsimd.memset(wgarb[:], 0.0)
        xgarb = accp.tile([96, 2, BM], fp8, tag="xgarb")
        nc.gpsimd.memset(xgarb[:], 0.0)

        # Schraudolph int16 outputs, flat: [512 block0 | per-D-super widths]
        d_widths = [512 if W == 512 else W for W, c in zip(SUPER_WIDTHS, ASSIGN) if c == "D"]
        sxbuf = sp.tile([BM, sum(d_widths)], i16, tag="sxbuf")
        # GPS fold outputs (bf16 sums of half-pairs), flat per big D super
        gbuf = gp.tile([BM, sum(p[1] // 2 for p in PAIRS)], bf16, tag="gbuf")
        # single 8-bank PSUM tile: window split varies per pair; A at [0:wA],
        # D at [wA:4096] -- all boundaries 512-aligned
        psbig = pp.tile([BM, 4096], f32, tag="psbig")

        offs = [sum(SUPER_WIDTHS[:k]) for k in range(len(SUPER_WIDTHS))]
        wtiles = {}
        # first A-super's weights land first so ACT starts ~1.4us earlier;
        # block0 (DVE) second; stream order afterwards
        for j in [1, 0]:
            wtile_pre = wp.tile([96, 2, 2048], fp8, tag="w")
            wtiles[j] = wtile_pre
            nc.sync.dma_start(wtile_pre[:, :, : SUPER_WIDTHS[j]],
                              wt[:, :, offs[j] : offs[j] + SUPER_WIDTHS[j]])

        ia = 0
        sx_off = 0
        g_off = 0
        folds = []      # (g_lo, width) per folded region, in gbuf order
        for j, W in enumerate(SUPER_WIDTHS):
            eng = ASSIGN[j]
            if j in wtiles:
                wtile = wtiles[j]
            else:
                wtile = wp.tile([96, 2, 2048], fp8, tag="w")
                nc.sync.dma_start(wtile[:, :, :W], wt[:, :, offs[j] : offs[j] + W])
            ps = psbig[:, (j % 2) * 2048 : (j % 2) * 2048 + W]
            for b in range(0, W, 512):
                nc.tensor.matmul(
                    ps[:, b : b + 512],
                    xtile[:],
                    wtile[:, :, b : b + 512],
                    start=True,
                    stop=True,
                    perf_mode=mybir.MatmulPerfMode.DoubleRow,
                )
            # two half-width consumers: the first frees its PSUM half early so
            # the next same-engine super's matmuls overlap the second half
            h = W // 2
            if eng == "A":
                ex = ep.tile([BM, 2048], bf16, tag="ex")
                for hi in range(2):
                    nc.scalar.activation(
                        ex[:, hi * h : hi * h + h],
                        ps[:, hi * h : hi * h + h],
                        AF.Exp,
                        bias=0.0,
                        scale=ACT_SCALE,
                        accum_out=acc[:, ia : ia + 1],
                    )
                    ia += 1
            else:
                for hi in range(2):
                    nc.vector.tensor_scalar(
                        sxbuf[:, sx_off + hi * h : sx_off + hi * h + h],
                        ps[:, hi * h : hi * h + h],
                        SCH_A_EFF,
                        SCH_B,
                        op0=mybir.AluOpType.mult,
                        op1=mybir.AluOpType.add,
                    )
                sx_off += W
        # DVE reduces: block0's raw sx (512), then gbuf in two contiguous runs
        red_plan = [("sx", 0, 512, 0), ("sx", 512, 4096, 1),
                    ("sx", 4608, 4096, 2), ("sx", 8704, 2048, 3)]
        for kind, lo, width, col in red_plan:
            srcap = (sxbuf[:, lo : lo + width].bitcast(bf16) if kind == "sx"
                     else gbuf[:, lo : lo + width])
            dd = dp.tile([BM, 4096], bf16, tag="dd")
            nc.vector.tensor_scalar(
                dd[:, :width],
                srcap,
                1.0,
                0.0,
                op0=mybir.AluOpType.mult,
                op1=mybir.AluOpType.add,
                accum_out=acc[:, n_act + col : n_act + col + 1],
            )

        nc.sync.dma_start(out, acc[:])

    nc.compile()
    return nc


def _get_nc():
    if DTYPE not in _CACHE:
        _CACHE[DTYPE] = _build()
    return _CACHE[DTYPE]


def _l2n(x, axis=-1):
    n = np.linalg.norm(x.astype(np.float32), axis=axis, keepdims=True)
    return x / np.maximum(n, 1e-12)


def _device_sumexp(xn, wn, trace=False):
    """Run the 8-core SPMD kernel. xn: [BM, D] f32 normalized rows;
    wn: [NC, D] f32 normalized rows. Returns sum_full [BM] f64 in the
    exp(SCALE*cos - SCALE) convention."""
    global LAST_EXEC_NS, LAST_RESULTS
    import ml_dtypes

    fp8 = np.dtype(ml_dtypes.float8_e4m3)
    xq = np.ascontiguousarray(
        (xn.T * PRE).reshape(2, 96, BM).swapaxes(0, 1)
    ).astype(fp8)                                          # [96, 2, 128]
    in_maps = []
    for k in range(N_CORES):
        sl = wn[k * C_SH : (k + 1) * C_SH]                 # [25000, D]
        wq = np.zeros((96, 2, C_PAD), fp8)
        wq[:, :, :C_SH] = (
            (sl.T * PRE).reshape(2, 96, C_SH).swapaxes(0, 1).astype(fp8)
        )
        in_maps.append({"wt": wq, "xt": xq})
    import os as _os

    _os.environ.setdefault("BASS_NEVER_TRACE", "1")
    nc = _get_nc()
    res = None
    last_err = None
    for attempt in range(3):
        try:
            res = run_bass_kernel_spmd(
                nc, in_maps, core_ids=list(range(N_CORES)), trace=trace
            )
            break
        except Exception as e:  # wedged-device NRT errors recover on retry
            last_err = e
            import time as _time

            _time.sleep(2.0)
    if res is None:
        raise last_err
    LAST_EXEC_NS = res.exec_time_ns
    LAST_RESULTS = res
    total = np.zeros(BM, np.float64)
    for k in range(N_CORES):
        accs = res.results[k]["out"].astype(np.float64)
        total += accs.reshape(BM, -1).sum(axis=1)
    # padded zero cols live in an ACT-exact super: exp(0) = 1.0 each; then
    # shift from e^(30c) to the e^(30c-30) convention
    total -= PAD_SUB
    return total * math.exp(-SCALE)


def kernel(pred_embs, pred_ps, gt_labels, weight):
    pred_embs = np.asarray(pred_embs, dtype=np.float32)
    pred_ps = np.asarray(pred_ps, dtype=np.float32)
    gt_labels = np.asarray(gt_labels)
    weight = np.asarray(weight, dtype=np.float32)

    # --- host marshalling: l2 normalize both operands (f32, like the ref) ---
    x = pred_embs.reshape(BM, D)
    xn = _l2n(x)                                           # [128, 192]
    wn = _l2n(weight)                                      # [200000, 192]

    # --- device: all-class sum of exp(30*cos - 30), sharded over 8 cores ---
    sum_full = _device_sumexp(xn, wn)                      # [128] f64
    sum_full = sum_full.reshape(B, M)

    # --- host: labels, mirroring jax.lax.top_k(gt_labels, S_SPK)[1]
    labels = np.argsort(-gt_labels, axis=1, kind="stable")[:, :S_SPK]

    # --- host: exact cos at label columns (128 rows of W) ---
    xn64 = xn.reshape(B, M, D).astype(np.float64)
    wl = _l2n(weight[labels]).astype(np.float64)           # [B, S, D]
    cos_l = np.einsum("bmd,bsd->bms", xn64, wl)            # [B, M, S]

    sin_l = np.sqrt(np.clip(1.0 - cos_l**2, 0.0, 1.0))
    phi_l = cos_l * COS_M - sin_l * SIN_M
    phi_l = np.where(cos_l > TH, phi_l, cos_l - MM)

    # logsumexp with the label column replaced by phi (shift = SCALE)
    adj = (
        sum_full[:, :, None]
        - np.exp(SCALE * cos_l - SCALE)
        + np.exp(SCALE * phi_l - SCALE)
    )
    lse = SCALE + np.log(adj)                              # [B, M, S]
    ce = lse - SCALE * phi_l
    C = np.swapaxes(ce, 1, 2)                              # [B, S, M]

    # Hungarian on 4x4 via brute force over 24 permutations
    import itertools

    perms = np.array(list(itertools.permutations(range(S_SPK))), np.int64)
    pc = C[:, np.arange(S_SPK)[None, :], perms].sum(-1)    # [B, P]
    best = np.argmin(pc, axis=1)
    col = perms[best]                                      # [B, S]

    matched = C[np.arange(B)[:, None], np.arange(S_SPK)[None, :], col]
    L_spk = matched.mean(axis=1)                           # [B]

    t_exist = np.zeros((B, M), np.float64)
    t_exist[np.arange(B)[:, None], col] = 1.0
    p = np.clip(pred_ps.astype(np.float64), EPS, 1.0 - EPS)
    L_exist = -(t_exist * np.log(p) + (1.0 - t_exist) * np.log(1.0 - p)).mean(axis=1)
    L_stop = -np.log(np.clip(pred_ps[:, -1].astype(np.float64), EPS, 1.0 - EPS))

    L_total = 0.01 * L_spk + ETA * L_exist + XI * L_stop
    return (
        np.float32(L_total.mean()),
        np.float32(L_spk.mean()),
        np.float32(L_exist.mean()),
        np.float32(L_stop.mean()),
    )
